# revision 1
# baseline (speedup 1.0000x reference)
"""Trainium2 Bass kernel for nn_MEGANCore (GATv2-style message-passing GNN).

Key insight 1: in the reference, _gatv2 gathers x_j = xp[col] and segment-sums
x_j * alpha by col; softmax weights alpha sum to 1 within each segment (and
self-loops guarantee non-empty segments), so the aggregation is exactly
xp = h @ W: the edges never matter.  The network collapses to a per-node
linear chain + layernorms + pooling + MLP.

Key insight 2 (folding): with ln_bias == 0 (asserted), each layer is
    h_{l+1} = rstd_l * (h_l @ B_l),   B_l = diag(scale_{l-1}) (I + (W0+W1)/2) C
with C = I - 11^T/64 the centering matrix and rstd a per-node scalar.
Per-node scalars commute through the chain; dropping the LN eps=1e-5 inside
the chain (verified 3e-6 absmax-relative on the final output) the scalars
all cancel except a final c4 = 1/sqrt(mean((x @ B*)^2)) with
B* = B0@B1@B2@B3 precomputed on host.  The device computes only:

    h~ = x @ B*                    (one 64x64 matmul per 128-node block)
    c4 = rsqrt(mean(h~^2, feat))   (per node)
    g  = (Mpool * c4)^T @ h~       (pooling, 8 graphs/core)
    out = relu(g@W1'+b1)@W2+b2     (W1' = diag(ln_scale[3]) @ W1)

Sharding: batch is sorted; 64 graphs -> 8 graphs per core, contiguous node
ranges padded to NPAD.  Host prep is pure data layout (transpose/pad/
one-hot/weight folding).  Matmuls run as float32r (full fp32 storage,
fast PE mode; measured 2.3e-4 absmax-relative error), fp32 statistics.
"""

import numpy as np

HID = 64
NCORES = 8
GPC = 8                 # graphs per core
NBLK = 52               # 128-node blocks per core
NPAD = NBLK * 128       # 6656 padded nodes per core
QB = 13                 # blocks per psum quarter
EPS_SQ = 1e-9           # guards rsqrt on zero-padded nodes

_prog = None


def _build_program():
    import concourse.tile as tile
    from concourse import bacc, mybir
    from contextlib import ExitStack

    f32 = mybir.dt.float32
    f32r = mybir.dt.float32r
    bf16 = mybir.dt.bfloat16

    nc = bacc.Bacc(
        "TRN2", target_bir_lowering=False, debug=False, num_devices=NCORES
    )
    xT = nc.dram_tensor("xT", [64, NPAD], f32r, kind="ExternalInput").ap()
    Bs = nc.dram_tensor("Bs", [64, 64], f32r, kind="ExternalInput").ap()
    Mp = nc.dram_tensor("Mp", [128, NBLK * GPC], f32, kind="ExternalInput").ap()
    W1 = nc.dram_tensor("W1", [64, 32], f32r, kind="ExternalInput").ap()
    b1 = nc.dram_tensor("b1", [32, 1], f32, kind="ExternalInput").ap()
    W2 = nc.dram_tensor("W2", [32, 1], f32r, kind="ExternalInput").ap()
    b2 = nc.dram_tensor("b2", [1, 1], f32, kind="ExternalInput").ap()
    ey = nc.dram_tensor("ey", [8, 8], f32, kind="ExternalInput").ap()
    out = nc.dram_tensor("out", [1, GPC], f32, kind="ExternalOutput").ap()

    with tile.TileContext(nc) as tc:
        with ExitStack() as ctx:
            _body(ctx, tc, nc, mybir, xT, Bs, Mp, W1, b1, W2, b2, ey, out)
    nc.compile()
    return nc


def _body(ctx, tc, nc, mybir, xT, Bs, Mp, W1, b1, W2, b2, ey, out):
    f32 = mybir.dt.float32
    f32r = mybir.dt.float32r
    bf16 = mybir.dt.bfloat16
    AF = mybir.ActivationFunctionType
    AX = mybir.AxisListType
    ALU = mybir.AluOpType

    const = ctx.enter_context(tc.tile_pool(name="const", bufs=1))
    spool = ctx.enter_context(tc.tile_pool(name="scr", bufs=1))
    xpool = ctx.enter_context(tc.tile_pool(name="xp", bufs=1))
    l3p = ctx.enter_context(tc.tile_pool(name="l3p", bufs=2, space="PSUM"))
    gps = ctx.enter_context(tc.tile_pool(name="gps", bufs=1, space="PSUM"))

    Bsb = const.tile([64, 64], f32r, tag="Bsb")
    nc.sync.dma_start(Bsb[:], Bs)
    Mpsb = const.tile([128, NBLK * GPC], f32, tag="Mpsb")
    nc.sync.dma_start(Mpsb[:], Mp)
    W1sb = const.tile([64, 32], f32r, tag="W1sb")
    nc.sync.dma_start(W1sb[:], W1)
    b1sb = const.tile([32, 1], f32, tag="b1sb")
    nc.sync.dma_start(b1sb[:], b1)
    W2sb = const.tile([32, 1], f32r, tag="W2sb")
    nc.sync.dma_start(W2sb[:], W2)
    b2sb = const.tile([1, 1], f32, tag="b2sb")
    nc.sync.dma_start(b2sb[:], b2)
    eysb = const.tile([8, 8], f32, tag="eysb")
    nc.sync.dma_start(eysb[:], ey)
    epsb = const.tile([128, 1], f32, tag="epsb")
    nc.vector.memset(epsb[:], EPS_SQ)

    # ---- load x (feat-major, host-transposed), per-quarter chunks ----
    xsb = xpool.tile([64, NPAD], f32r, tag="xsb")
    for q in range(4):
        nc.sync.dma_start(
            xsb[:, q * QB * 128:(q + 1) * QB * 128],
            xT[:, q * QB * 128:(q + 1) * QB * 128],
        )

    # ---- h~ = x @ B* per 128-node block (node-major out), stats, evict ----
    y3 = spool.tile([128, NBLK * 64], f32r, tag="y3")
    sq = spool.tile([128, NBLK * 64], f32, tag="sq")
    msq = spool.tile([128, NBLK], f32, tag="msq")
    for q in range(4):
        ps = l3p.tile([128, QB * 64], f32, tag="l3")
        for i in range(QB):
            t = q * QB + i
            nc.tensor.matmul(
                ps[:, i * 64:(i + 1) * 64],
                xsb[:, t * 128:(t + 1) * 128],
                Bsb[:],
                start=True, stop=True,
            )
        half = QB * 64 // 2  # split eviction DVE/ACT
        q0 = q * QB * 64
        nc.vector.tensor_copy(y3[:, q0:q0 + half], ps[:, :half])
        nc.scalar.copy(y3[:, q0 + half:q0 + QB * 64], ps[:, half:])
        nc.scalar.square(sq[:, q0:q0 + QB * 64], ps[:])
        nc.vector.tensor_reduce(
            msq[:, q * QB:(q + 1) * QB],
            sq[:, q0:q0 + QB * 64].rearrange("p (b f) -> p b f", f=64),
            axis=AX.X, op=ALU.add,
        )

    # ---- c4 = 1/sqrt(msq/64 + eps), folded into pooling weights ----
    c4a = spool.tile([128, NBLK], f32, tag="c4a")
    nc.scalar.activation(c4a[:], msq[:], AF.Sqrt, bias=epsb[:], scale=1.0 / 64)
    c4 = spool.tile([128, NBLK], f32, tag="c4")
    nc.vector.reciprocal(c4[:], c4a[:])

    mp2 = spool.tile([128, NBLK * GPC], f32r, tag="mp2")
    for t in range(NBLK):
        nc.vector.tensor_scalar_mul(
            mp2[:, t * GPC:(t + 1) * GPC],
            Mpsb[:, t * GPC:(t + 1) * GPC],
            c4[:, t:t + 1],
        )

    # ---- pooling: g[8,64] = sum_t (Mpool*c4)[:,t]^T @ y3[:,t] ----
    g = gps.tile([8, 64], f32, tag="gmlp")
    for t in range(NBLK):
        nc.tensor.matmul(
            g[:],
            mp2[:, t * GPC:(t + 1) * GPC],
            y3[:, t * 64:(t + 1) * 64],
            start=(t == 0), stop=(t == NBLK - 1),
        )

    # ---- MLP head ----
    gsb = spool.tile([8, 64], f32, tag="gsb")
    nc.vector.tensor_copy(gsb[:], g[:])
    gT = gps.tile([64, 8], f32, tag="gmlp")
    nc.tensor.transpose(gT[:], gsb[:], eysb[:])
    gTsb = spool.tile([64, 8], f32r, tag="gTsb")
    nc.vector.tensor_copy(gTsb[:], gT[:])
    hid = gps.tile([32, 8], f32, tag="gmlp")
    nc.tensor.matmul(hid[:], W1sb[:], gTsb[:], start=True, stop=True)
    hsb = spool.tile([32, 8], f32r, tag="hsb")
    nc.scalar.activation(hsb[:], hid[:], AF.Relu, bias=b1sb[:, 0:1], scale=1.0)
    o = gps.tile([1, 8], f32, tag="gmlp")
    nc.tensor.matmul(o[:], W2sb[:], hsb[:], start=True, stop=True)
    osb = spool.tile([1, 8], f32, tag="osb")
    nc.scalar.activation(osb[:], o[:], AF.Identity, bias=b2sb[:, 0:1], scale=1.0)
    nc.sync.dma_start(out, osb[:])


def _prep_inputs(inputs):
    import ml_dtypes

    x = np.ascontiguousarray(np.asarray(inputs["x"], dtype=np.float32))
    batch = np.asarray(inputs["batch"]).astype(np.int64)
    Wn = np.asarray(inputs["Wn"], dtype=np.float32)
    ln_scale = np.asarray(inputs["ln_scale"], dtype=np.float32)
    ln_bias = np.asarray(inputs["ln_bias"], dtype=np.float32)
    W1 = np.asarray(inputs["W1"], dtype=np.float32)
    b1 = np.asarray(inputs["b1"], dtype=np.float32)
    W2 = np.asarray(inputs["W2"], dtype=np.float32)
    b2 = np.asarray(inputs["b2"], dtype=np.float32)
    assert np.allclose(ln_bias, 0.0), "kernel assumes ln_bias == 0"

    C = (np.eye(HID) - np.ones((HID, HID)) / HID).astype(np.float32)
    Bstar = np.eye(HID, dtype=np.float32)
    for l in range(4):
        A = np.eye(HID, dtype=np.float32) + (Wn[l, 0] + Wn[l, 1]) * 0.5
        S = (
            np.diag(ln_scale[l - 1]).astype(np.float32)
            if l > 0 else np.eye(HID, dtype=np.float32)
        )
        Bstar = Bstar @ (S @ A @ C)
    Bstar = np.ascontiguousarray(Bstar.astype(np.float32))
    W1p = np.ascontiguousarray(
        (np.diag(ln_scale[3]).astype(np.float32) @ W1).astype(np.float32)
    )

    bounds = np.searchsorted(batch, np.arange(0, 65, GPC))
    in_maps = []
    for c in range(NCORES):
        s, e = int(bounds[c]), int(bounds[c + 1])
        n = e - s
        assert n <= NPAD, f"core {c} shard {n} > NPAD {NPAD}"
        xTc = np.zeros((64, NPAD), dtype=np.float32)
        xTc[:, :n] = x[s:e].T
        mp = np.zeros((128, NBLK * GPC), dtype=np.float32)
        gb = (batch[s:e] - GPC * c).astype(np.int64)
        idx = np.arange(n)
        mp[idx % 128, (idx // 128) * GPC + gb] = 1.0
        in_maps.append(
            dict(
                xT=xTc,
                Bs=Bstar,
                Mp=np.ascontiguousarray(mp),
                W1=W1p,
                b1=np.ascontiguousarray(b1.reshape(32, 1)),
                W2=np.ascontiguousarray(W2.reshape(32, 1)),
                b2=np.ascontiguousarray(b2.reshape(1, 1)),
                ey=np.eye(8, dtype=np.float32),
            )
        )
    return in_maps


def kernel(**inputs):
    global _prog
    from concourse import bass_utils

    in_maps = _prep_inputs(inputs)
    if _prog is None:
        _prog = _build_program()
    res = bass_utils.run_bass_kernel_spmd(
        _prog, in_maps, core_ids=list(range(NCORES))
    )
    outs = [np.asarray(res.results[c]["out"]).reshape(GPC) for c in range(NCORES)]
    return np.concatenate(outs).reshape(64, 1).astype(np.float32)



# revision 5
# speedup vs baseline: 1.6741x; 1.6741x over previous
"""Trainium2 Bass kernel for nn_MEGANCore (GATv2-style message-passing GNN).

Algebraic collapse (same as prior version): the reference's _gatv2 gathers
x_j = xp[col] and segment-sums x_j * alpha by col; softmax weights sum to 1
per segment, so aggregation == xp and the edges never matter.  With
ln_bias == 0 the 4-layer chain folds into one matrix B* (host-precomputed);
per-node LN scalars cancel except a final c4 = rsqrt(mean((x @ B*)^2)).
Since pooling is linear, g_b = (sum_n c4_n x_n) @ B*, so the device computes

    sumsq_n = ||x_n @ B*||^2        (A-phase + square + reduce)
    c4_n    = rsqrt(sumsq_n/64+eps)
    g0      = sum_n c4_n x_n        (pooling over raw x, per graph)
    out     = relu((g0@B*)@W1'+b1)@W2+b2

Device mapping (all x traffic bf16, ~0.85 MB per layout copy per core):
  A-phase : stationary block-diag [[B*,0],[0,B*]] (one FWL load), stream
            pair-major xT2[128, 3328] -> h~ for 2 nodes/cycle, PSUM [128,512]
  square  : PSUM->SBUF eviction split ACT/DVE, bf16 out
  reduce  : sq 128-col blocks as FWL weights x even/odd ones mask [128,16]
            -> per-pair sumsq lands node-major (transpose+reduce in one MM)
  pooling : xPW 128-col blocks as FWL weights x c4-weighted one-hot Q
            -> g0^T accumulated in PSUM [128,16] (even/odd feature halves)
  head    : two accumulating MMs on row-groups fold even/odd, then tiny MLP
  warmup  : dummy MMs + dummy activations during the DMA wait keep the PE
            HAM-warm (2.4 GHz) and hoist ACT table loads off the hot path
"""

import numpy as np

HID = 64
NCORES = 8
GPC = 8                  # graphs per core
NPAD = 6656              # padded nodes per core
P = NPAD // 2            # 3328 node-pairs
PBLK = P // 128          # 26 pair-blocks
CHUNK = 512              # pairs per A-phase matmul
NCH = (P + CHUNK - 1) // CHUNK   # 7 chunks (6x512 + 256)
EPS_SQ = 1e-9

_prog = None


def _build_program():
    import concourse.tile as tile
    from concourse import bacc, mybir
    from contextlib import ExitStack

    f32 = mybir.dt.float32
    f32r = mybir.dt.float32r
    bf16 = mybir.dt.bfloat16

    nc = bacc.Bacc(
        "TRN2", target_bir_lowering=False, debug=False, num_devices=NCORES
    )
    xT2 = nc.dram_tensor("xT2", [128, P], bf16, kind="ExternalInput").ap()
    xPW = nc.dram_tensor("xPW", [128, PBLK * 128], bf16, kind="ExternalInput").ap()
    cb = nc.dram_tensor("cb", [128, 560], bf16, kind="ExternalInput").ap()
    cf = nc.dram_tensor("cf", [64, 35], f32r, kind="ExternalInput").ap()
    out = nc.dram_tensor("out", [1, GPC], f32, kind="ExternalOutput").ap()

    with tile.TileContext(nc) as tc:
        with ExitStack() as ctx:
            _body(ctx, tc, nc, mybir, xT2, xPW, cb, cf, out)
    nc.compile()
    return nc


def _body(ctx, tc, nc, mybir, xT2, xPW, cb, cf, out):
    f32 = mybir.dt.float32
    f32r = mybir.dt.float32r
    bf16 = mybir.dt.bfloat16
    AF = mybir.ActivationFunctionType

    const = ctx.enter_context(tc.tile_pool(name="const", bufs=1))
    spool = ctx.enter_context(tc.tile_pool(name="scr", bufs=1))
    sqt = ctx.enter_context(tc.tile_pool(name="sqt", bufs=3))
    aps = ctx.enter_context(tc.tile_pool(name="aps", bufs=3, space="PSUM"))
    wps = ctx.enter_context(tc.tile_pool(name="wps", bufs=1, space="PSUM"))
    cpsp = ctx.enter_context(tc.tile_pool(name="cps", bufs=1, space="PSUM"))
    bpsp = ctx.enter_context(tc.tile_pool(name="bps", bufs=1, space="PSUM"))
    gps = ctx.enter_context(tc.tile_pool(name="gps", bufs=1, space="PSUM"))

    # ---- local scratch (no DMA deps) ----
    epsb = const.tile([128, 1], f32, tag="epsb")
    nc.vector.memset(epsb[:], EPS_SQ)
    scr = const.tile([128, 512], bf16, tag="scr")
    nc.vector.memset(scr[:], 0.0)
    dumm = const.tile([1, 4], f32, tag="dumm")
    nc.vector.memset(dumm[:], 1.0)

    # ---- input DMAs (few, large, issued on separate engines) ----
    xasb = spool.tile([128, P], bf16, tag="xasb")
    nc.sync.dma_start(xasb[:], xT2)
    xbsb = spool.tile([128, PBLK * 128], bf16, tag="xbsb")
    nc.scalar.dma_start(xbsb[:], xPW)
    cbsb = const.tile([128, 560], bf16, tag="cbsb")
    nc.gpsimd.dma_start(cbsb[:], cb)
    cfsb = const.tile([64, 35], f32r, tag="cfsb")
    nc.gpsimd.dma_start(cfsb[:], cf)
    BD = cbsb[:, 0:128]
    Mp = cbsb[:, 128:544]
    EO = cbsb[:, 544:560]

    # ---- ACT table warm (hoists ACT_TABLE_LOAD off the critical path) ----
    dto = const.tile([1, 4], f32, tag="dto")
    nc.scalar.activation(dto[:, 0:1], dumm[:1, 0:1], AF.Square)
    nc.scalar.activation(dto[:, 1:2], dumm[:1, 1:2], AF.Sqrt, bias=epsb[0:1, :])
    nc.scalar.activation(dto[:, 2:3], dumm[:1, 2:3], AF.Relu, bias=epsb[0:1, :])
    nc.scalar.activation(dto[:, 3:4], dumm[:1, 3:4], AF.Identity, bias=epsb[0:1, :])

    # ---- PE HAM warmup: junk matmuls on zeroed scratch during DMA wait ----
    wp = wps.tile([128, 512], f32, tag="warm")
    for _ in range(6):
        nc.tensor.matmul(wp[:], scr[:, 0:128], scr[:], start=True, stop=True)

    sq = spool.tile([128, P], bf16, tag="sq")
    c4a = spool.tile([128, PBLK * 16], f32, tag="c4a")
    c4r = spool.tile([128, PBLK * 16], f32, tag="c4r")
    Q = spool.tile([128, PBLK * 16], bf16, tag="Q")
    cps = cpsp.tile([128, PBLK * 16], f32, tag="cps")
    bp1 = bpsp.tile([128, 16], f32, tag="bp1")
    bp2 = bpsp.tile([128, 16], f32, tag="bp2")

    achunks = []
    for c in range(NCH):
        c0 = c * CHUNK
        w = min(CHUNK, P - c0)
        achunks.append((c0, w))

    def emit_A(c):
        c0, w = achunks[c]
        ps = aps.tile([128, 512], f32, tag="aps")
        nc.tensor.matmul(ps[:, 0:w], BD, xasb[:, c0:c0 + w], start=True, stop=True)
        return ps

    def emit_sq(c, ps):
        c0, w = achunks[c]
        aw = 3 * w // 4          # ACT squares 3/4 straight from PSUM
        dw = w - aw              # DVE: copy-out then square in bf16
        nc.scalar.activation(sq[:, c0:c0 + aw], ps[:, 0:aw], AF.Square)
        t = sqt.tile([128, 128], bf16, tag="sqt")
        nc.vector.tensor_copy(t[:, 0:dw], ps[:, aw:w])
        nc.vector.tensor_mul(sq[:, c0 + aw:c0 + w], t[:, 0:dw], t[:, 0:dw])

    def emit_reduce(b):
        nc.tensor.matmul(
            cps[:, b * 16:(b + 1) * 16],
            sq[:, b * 128:(b + 1) * 128], EO,
            start=True, stop=True,
        )

    def emit_c4(h):  # half h in {0, 1}: blocks 0..12 / 13..25
        lo = 0 if h == 0 else 13 * 16
        hi = 13 * 16 if h == 0 else PBLK * 16
        nc.scalar.activation(
            c4a[:, lo:hi], cps[:, lo:hi], AF.Sqrt, bias=epsb[:], scale=1.0 / 64
        )
        nc.vector.reciprocal_approx_fast(c4r[:, lo:hi], c4a[:, lo:hi])
        nc.vector.tensor_mul(Q[:, lo:hi], Mp[:, lo:hi], c4r[:, lo:hi])

    def emit_B(b, ps, first, last):
        nc.tensor.matmul(
            ps[:], xbsb[:, b * 128:(b + 1) * 128], Q[:, b * 16:(b + 1) * 16],
            start=first, stop=last,
        )

    # ---- software-pipelined emission ----
    pend = {}
    pend[0] = emit_A(0)
    pend[1] = emit_A(1)
    emit_sq(0, pend.pop(0))
    for c in range(2, NCH):
        pend[c] = emit_A(c)
        emit_sq(c - 1, pend.pop(c - 1))
        for b in range(4 * (c - 2), 4 * (c - 1)):
            emit_reduce(b)
    # chunks emitted: A all; sq through NCH-2; reduce through block 4*(NCH-2)-1=19
    emit_sq(NCH - 1, pend.pop(NCH - 1))
    for b in range(4 * (NCH - 2), 4 * (NCH - 2) + 2):   # blocks 20, 21
        emit_reduce(b)
    emit_c4(0)                                          # needs blocks 0..12
    for b in range(0, 13):
        emit_B(b, bp1, b == 0, b == 12)
    for b in range(22, PBLK):                           # blocks 22..25
        emit_reduce(b)
    emit_c4(1)
    for b in range(13, PBLK):
        emit_B(b, bp2, b == 13, b == PBLK - 1)

    # ---- combine even/odd pooled halves: g0sb = bp1 + bp2 ----
    g0t = spool.tile([128, 16], bf16, tag="g0t")
    nc.vector.tensor_copy(g0t[:], bp1[:])
    g0sb = spool.tile([128, 16], bf16, tag="g0sb")
    nc.vector.tensor_add(g0sb[:], g0t[:], bp2[:])

    # ---- head: gT = B*^T (g0Te + g0To), then MLP ----
    gt = gps.tile([64, 8], f32, tag="gmlp")
    nc.tensor.matmul(gt[:], cbsb[0:64, 0:64], g0sb[0:64, 0:8],
                     start=True, stop=False)
    nc.tensor.matmul(gt[:], cbsb[64:128, 64:128], g0sb[64:128, 8:16],
                     start=False, stop=True)
    gsb = spool.tile([64, 8], f32r, tag="gsb")
    nc.vector.tensor_copy(gsb[:], gt[:])
    hid = gps.tile([32, 8], f32, tag="gmlp")
    nc.tensor.matmul(hid[:], cfsb[:, 0:32], gsb[:], start=True, stop=True)
    hsb = spool.tile([32, 8], f32r, tag="hsb")
    nc.scalar.activation(hsb[:], hid[:], AF.Relu, bias=cfsb[0:32, 32:33])
    o = gps.tile([1, 8], f32, tag="gmlp")
    nc.tensor.matmul(o[:], cfsb[0:32, 33:34], hsb[:], start=True, stop=True)
    osb = spool.tile([1, 8], f32, tag="osb")
    nc.scalar.activation(osb[:], o[:], AF.Identity, bias=cfsb[0:1, 34:35])
    nc.sync.dma_start(out, osb[:])


def _prep_inputs(inputs):
    import ml_dtypes

    bf16 = ml_dtypes.bfloat16
    x = np.ascontiguousarray(np.asarray(inputs["x"], dtype=np.float32))
    batch = np.asarray(inputs["batch"]).astype(np.int64)
    Wn = np.asarray(inputs["Wn"], dtype=np.float32)
    ln_scale = np.asarray(inputs["ln_scale"], dtype=np.float32)
    ln_bias = np.asarray(inputs["ln_bias"], dtype=np.float32)
    W1 = np.asarray(inputs["W1"], dtype=np.float32)
    b1 = np.asarray(inputs["b1"], dtype=np.float32)
    W2 = np.asarray(inputs["W2"], dtype=np.float32)
    b2 = np.asarray(inputs["b2"], dtype=np.float32)
    assert np.allclose(ln_bias, 0.0), "kernel assumes ln_bias == 0"

    C = (np.eye(HID) - np.ones((HID, HID)) / HID).astype(np.float32)
    Bstar = np.eye(HID, dtype=np.float32)
    for l in range(4):
        A = np.eye(HID, dtype=np.float32) + (Wn[l, 0] + Wn[l, 1]) * 0.5
        S = (
            np.diag(ln_scale[l - 1]).astype(np.float32)
            if l > 0 else np.eye(HID, dtype=np.float32)
        )
        Bstar = Bstar @ (S @ A @ C)
    Bstar = Bstar.astype(np.float32)
    W1p = (np.diag(ln_scale[3]).astype(np.float32) @ W1).astype(np.float32)

    BD = np.zeros((128, 128), np.float32)
    BD[0:64, 0:64] = Bstar
    BD[64:128, 64:128] = Bstar
    EO = np.zeros((128, 16), np.float32)
    EO[0:64, 0:8] = 1.0
    EO[64:128, 8:16] = 1.0
    cf = np.zeros((64, 35), np.float32)
    cf[:, 0:32] = W1p
    cf[0:32, 32] = b1
    cf[0:32, 33] = W2[:, 0]
    cf[0, 34] = b2[0]
    cf = np.ascontiguousarray(cf)

    bounds = np.searchsorted(batch, np.arange(0, 65, GPC))
    in_maps = []
    for c in range(NCORES):
        s, e = int(bounds[c]), int(bounds[c + 1])
        n = e - s
        assert n <= NPAD, f"core {c} shard {n} > NPAD {NPAD}"
        xp = np.zeros((NPAD, HID), np.float32)
        xp[:n] = x[s:e]
        xpr = xp.reshape(P, 2, HID)
        xT2 = np.concatenate([xpr[:, 0, :].T, xpr[:, 1, :].T], axis=0)
        xPW = (
            xpr.reshape(P, 128).reshape(PBLK, 128, 128)
            .transpose(1, 0, 2).reshape(128, PBLK * 128)
        )
        Mp = np.zeros((128, PBLK * 16), np.float32)
        i = np.arange(n)
        gb = (batch[s:e] - GPC * c).astype(np.int64)
        p = i // 2
        Mp[p % 128, (p // 128) * 16 + (i % 2) * 8 + gb] = 1.0
        cbm = np.concatenate([BD, Mp, EO], axis=1)
        in_maps.append(
            dict(
                xT2=np.ascontiguousarray(xT2.astype(bf16)),
                xPW=np.ascontiguousarray(xPW.astype(bf16)),
                cb=np.ascontiguousarray(cbm.astype(bf16)),
                cf=cf,
            )
        )
    return in_maps


def kernel(**inputs):
    global _prog
    from concourse import bass_utils

    in_maps = _prep_inputs(inputs)
    if _prog is None:
        _prog = _build_program()
    res = bass_utils.run_bass_kernel_spmd(
        _prog, in_maps, core_ids=list(range(NCORES))
    )
    outs = [np.asarray(res.results[c]["out"]).reshape(GPC) for c in range(NCORES)]
    return np.concatenate(outs).reshape(64, 1).astype(np.float32)


# revision 12
# speedup vs baseline: 1.7622x; 1.0526x over previous
"""Trainium2 Bass kernel for nn_MEGANCore (GATv2-style message-passing GNN).

Algebraic collapse (same as prior version): the reference's _gatv2 gathers
x_j = xp[col] and segment-sums x_j * alpha by col; softmax weights sum to 1
per segment, so aggregation == xp and the edges never matter.  With
ln_bias == 0 the 4-layer chain folds into one matrix B* (host-precomputed);
per-node LN scalars cancel except a final c4 = rsqrt(mean((x @ B*)^2)).
Since pooling is linear, g_b = (sum_n c4_n x_n) @ B*, so the device computes

    sumsq_n = ||x_n @ B*||^2        (A-phase + square + reduce)
    c4_n    = rsqrt(sumsq_n/64+eps)
    g0      = sum_n c4_n x_n        (pooling over raw x, per graph)
    out     = relu((g0@B*)@W1'+b1)@W2+b2

Device mapping (all x traffic bf16, ~0.85 MB per layout copy per core):
  A-phase : stationary block-diag [[B*,0],[0,B*]] (one FWL load), stream
            pair-major xT2[128, 3328] -> h~ for 2 nodes/cycle, PSUM [128,512]
  square  : PSUM->SBUF eviction split ACT/DVE, bf16 out
  reduce  : sq 128-col blocks as FWL weights x even/odd ones mask [128,16]
            -> per-pair sumsq lands node-major (transpose+reduce in one MM)
  pooling : xPW 128-col blocks as FWL weights x c4-weighted one-hot Q
            -> g0^T accumulated in PSUM [128,16] (even/odd feature halves)
  head    : two accumulating MMs on row-groups fold even/odd, then tiny MLP
  warmup  : dummy MMs + dummy activations during the DMA wait keep the PE
            HAM-warm (2.4 GHz) and hoist ACT table loads off the hot path
"""

import numpy as np

HID = 64
NCORES = 8
GPC = 8                  # graphs per core
NPAD = 6656              # padded nodes per core
P = NPAD // 2            # 3328 node-pairs
PBLK = P // 128          # 26 pair-blocks
CHUNK = 512              # pairs per A-phase matmul
NCH = (P + CHUNK - 1) // CHUNK   # 7 chunks (6x512 + 256)
EPS_SQ = 1e-9
FP8_A = True             # xT2 in fp8e4 (affects only the c4 stats path)
FP8_SQ = True            # squared activations in fp8e4 (halves reduce LDW)
NXDMA = 2                # xT2 arrives in this many column-chunks
# c4/pooling pieces (descending so the last piece's chain is short)
PIECES = [(0, 13), (13, 7), (20, 4), (24, 2)]   # (start block, nblocks)

_prog = None


def _build_program():
    import concourse.tile as tile
    from concourse import bacc, mybir
    from contextlib import ExitStack

    f32 = mybir.dt.float32
    f32r = mybir.dt.float32r
    bf16 = mybir.dt.bfloat16
    fp8 = mybir.dt.float8e4
    adt = fp8 if FP8_A else bf16

    nc = bacc.Bacc(
        "TRN2", target_bir_lowering=False, debug=False, num_devices=NCORES
    )
    xT2 = nc.dram_tensor("xT2", [128, P], adt, kind="ExternalInput").ap()
    xPW = nc.dram_tensor("xPW", [128, PBLK * 128], bf16, kind="ExternalInput").ap()
    cb = nc.dram_tensor("cb", [128, 560], bf16, kind="ExternalInput").ap()
    cb8 = nc.dram_tensor("cb8", [128, 144], fp8, kind="ExternalInput").ap()
    cf = nc.dram_tensor("cf", [64, 35], f32r, kind="ExternalInput").ap()
    out = nc.dram_tensor("out", [1, GPC], f32, kind="ExternalOutput").ap()

    with tile.TileContext(nc) as tc:
        with ExitStack() as ctx:
            _body(ctx, tc, nc, mybir, xT2, xPW, cb, cb8, cf, out)
    nc.compile()
    return nc


def _body(ctx, tc, nc, mybir, xT2, xPW, cb, cb8, cf, out):
    f32 = mybir.dt.float32
    f32r = mybir.dt.float32r
    bf16 = mybir.dt.bfloat16
    fp8 = mybir.dt.float8e4
    adt = fp8 if FP8_A else bf16
    sdt = fp8 if FP8_SQ else bf16
    AF = mybir.ActivationFunctionType

    const = ctx.enter_context(tc.tile_pool(name="const", bufs=1))
    spool = ctx.enter_context(tc.tile_pool(name="scr", bufs=1))
    sqt = ctx.enter_context(tc.tile_pool(name="sqt", bufs=3))
    aps = ctx.enter_context(tc.tile_pool(name="aps", bufs=3, space="PSUM"))
    wps = ctx.enter_context(tc.tile_pool(name="wps", bufs=1, space="PSUM"))
    cpsp = ctx.enter_context(tc.tile_pool(name="cps", bufs=1, space="PSUM"))
    bpsp = ctx.enter_context(tc.tile_pool(name="bps", bufs=1, space="PSUM"))
    gps = ctx.enter_context(tc.tile_pool(name="gps", bufs=1, space="PSUM"))

    # ---- local scratch (no DMA deps) ----
    epsb = const.tile([128, 1], f32, tag="epsb")
    nc.vector.memset(epsb[:], EPS_SQ)
    scr = const.tile([128, 512], bf16, tag="scr")
    nc.vector.memset(scr[:], 0.0)
    dumm = const.tile([1, 4], f32, tag="dumm")
    nc.vector.memset(dumm[:], 1.0)

    # ---- input DMAs: xT2 chunked on sync (critical path), consts on gpsimd,
    # ---- xPW on scalar AFTER the table-warm dummies so it trails xT2 ----
    xasb = spool.tile([128, P], adt, tag="xasb")
    xdw = P // NXDMA
    for d in range(NXDMA):
        nc.sync.dma_start(
            xasb[:, d * xdw:(d + 1) * xdw], xT2[:, d * xdw:(d + 1) * xdw]
        )
    cbsb = const.tile([128, 560], bf16, tag="cbsb")
    nc.gpsimd.dma_start(cbsb[:], cb)
    cb8sb = const.tile([128, 144], fp8, tag="cb8sb")
    nc.gpsimd.dma_start(cb8sb[:], cb8)
    cfsb = const.tile([64, 35], f32r, tag="cfsb")
    nc.gpsimd.dma_start(cfsb[:], cf)
    BD = cbsb[:, 0:128]
    Mp = cbsb[:, 128:544]
    EO = cb8sb[:, 0:16] if FP8_SQ else cbsb[:, 544:560]

    # ---- ACT table warm (hoists ACT_TABLE_LOAD off the critical path) ----
    dto = const.tile([1, 4], f32, tag="dto")
    nc.scalar.activation(dto[:, 0:1], dumm[:1, 0:1], AF.Square)
    nc.scalar.activation(dto[:, 1:2], dumm[:1, 1:2], AF.Sqrt, bias=epsb[0:1, :])
    nc.scalar.activation(dto[:, 2:3], dumm[:1, 2:3], AF.Relu, bias=epsb[0:1, :])
    nc.scalar.activation(dto[:, 3:4], dumm[:1, 3:4], AF.Identity, bias=epsb[0:1, :])

    xbsb = spool.tile([128, PBLK * 128], bf16, tag="xbsb")
    nc.scalar.dma_start(xbsb[:], xPW)

    # ---- PE HAM warmup: junk matmuls on zeroed scratch during DMA wait ----
    wp = wps.tile([128, 512], f32, tag="warm")
    for _ in range(5):
        nc.tensor.matmul(wp[:], scr[:, 0:128], scr[:], start=True, stop=True)

    sq = spool.tile([128, P], sdt, tag="sq")
    c4a = spool.tile([128, PBLK * 16], f32, tag="c4a")
    c4r = spool.tile([128, PBLK * 16], f32, tag="c4r")
    Q = spool.tile([128, PBLK * 16], bf16, tag="Q")
    cps = cpsp.tile([128, PBLK * 16], f32, tag="cps")
    bps = bpsp.tile([128, 16 * len(PIECES)], f32, tag="bps")

    achunks = []
    for c in range(NCH):
        c0 = c * CHUNK
        w = min(CHUNK, P - c0)
        achunks.append((c0, w))

    def emit_A(c):
        c0, w = achunks[c]
        ps = aps.tile([128, 512], f32, tag="aps")
        nc.tensor.matmul(ps[:, 0:w], BD, xasb[:, c0:c0 + w], start=True, stop=True)
        return ps

    def emit_sq(c, ps):
        c0, w = achunks[c]
        aw = 3 * w // 4          # ACT squares 3/4 straight from PSUM
        dw = w - aw              # DVE: copy-out then square
        nc.scalar.activation(sq[:, c0:c0 + aw], ps[:, 0:aw], AF.Square)
        t = sqt.tile([128, 128], bf16, tag="sqt")
        nc.vector.tensor_copy(t[:, 0:dw], ps[:, aw:w])
        nc.vector.tensor_mul(sq[:, c0 + aw:c0 + w], t[:, 0:dw], t[:, 0:dw])

    def emit_reduce(b):
        nc.tensor.matmul(
            cps[:, b * 16:(b + 1) * 16],
            sq[:, b * 128:(b + 1) * 128], EO,
            start=True, stop=True,
        )

    def emit_c4(pi):
        b0, nb = PIECES[pi]
        lo, hi = b0 * 16, (b0 + nb) * 16
        nc.scalar.activation(
            c4a[:, lo:hi], cps[:, lo:hi], AF.Sqrt, bias=epsb[:], scale=1.0 / 64
        )
        nc.vector.reciprocal_approx_fast(c4r[:, lo:hi], c4a[:, lo:hi])
        nc.vector.tensor_mul(Q[:, lo:hi], Mp[:, lo:hi], c4r[:, lo:hi])

    def emit_B(pi):
        b0, nb = PIECES[pi]
        t = bps[:, pi * 16:(pi + 1) * 16]
        for b in range(b0, b0 + nb):
            nc.tensor.matmul(
                t, xbsb[:, b * 128:(b + 1) * 128], Q[:, b * 16:(b + 1) * 16],
                start=(b == b0), stop=(b == b0 + nb - 1),
            )

    # ---- software-pipelined emission ----
    # chunk c covers reduce blocks 4c..4c+3 (last chunk: 2 blocks)
    pend = {}
    pend[0] = emit_A(0)
    pend[1] = emit_A(1)
    emit_sq(0, pend.pop(0))
    for c in range(2, NCH):
        pend[c] = emit_A(c)
        emit_sq(c - 1, pend.pop(c - 1))
        for b in range(4 * (c - 2), 4 * (c - 1)):
            emit_reduce(b)
        if c == NCH - 1:            # blocks 0..15 emitted; piece 0 = 0..12
            emit_c4(0)
            emit_B(0)
    emit_sq(NCH - 1, pend.pop(NCH - 1))
    for b in range(4 * (NCH - 2), 4 * (NCH - 2) + 4):   # blocks 20..23
        emit_reduce(b)
    emit_c4(1)                                          # blocks 13..19
    emit_B(1)
    for b in range(24, PBLK):                           # blocks 24, 25
        emit_reduce(b)
    emit_c4(2)                                          # blocks 20..23
    emit_B(2)
    emit_c4(3)                                          # blocks 24, 25
    emit_B(3)

    # ---- combine pooled pieces: g0sb = sum over the 4 psum slices ----
    bsb = spool.tile([128, 16 * len(PIECES)], f32, tag="bsb")
    nc.scalar.copy(bsb[:], bps[:])
    g0sb = spool.tile([128, 16], bf16, tag="g0sb")
    with nc.allow_low_precision(reason="4-way sum; result is bf16 regardless"):
        nc.vector.tensor_reduce(
            g0sb[:],
            bsb[:].rearrange("p (k s) -> p s k", s=16),
            axis=mybir.AxisListType.X, op=mybir.AluOpType.add,
        )

    # ---- head: gT = B*^T (g0Te + g0To), then MLP ----
    gt = gps.tile([64, 8], f32, tag="gmlp")
    nc.tensor.matmul(gt[:], cbsb[0:64, 0:64], g0sb[0:64, 0:8],
                     start=True, stop=False)
    nc.tensor.matmul(gt[:], cbsb[64:128, 64:128], g0sb[64:128, 8:16],
                     start=False, stop=True)
    gsb = spool.tile([64, 8], f32r, tag="gsb")
    nc.vector.tensor_copy(gsb[:], gt[:])
    hid = gps.tile([32, 8], f32, tag="gmlp")
    nc.tensor.matmul(hid[:], cfsb[:, 0:32], gsb[:], start=True, stop=True)
    hsb = spool.tile([32, 8], f32r, tag="hsb")
    nc.scalar.activation(hsb[:], hid[:], AF.Relu, bias=cfsb[0:32, 32:33])
    o = gps.tile([1, 8], f32, tag="gmlp")
    nc.tensor.matmul(o[:], cfsb[0:32, 33:34], hsb[:], start=True, stop=True)
    osb = spool.tile([1, 8], f32, tag="osb")
    nc.scalar.activation(osb[:], o[:], AF.Identity, bias=cfsb[0:1, 34:35])
    nc.sync.dma_start(out, osb[:])


def _prep_inputs(inputs):
    import ml_dtypes

    bf16 = ml_dtypes.bfloat16
    fp8 = ml_dtypes.float8_e4m3fn
    adt = fp8 if FP8_A else bf16
    x = np.ascontiguousarray(np.asarray(inputs["x"], dtype=np.float32))
    batch = np.asarray(inputs["batch"]).astype(np.int64)
    Wn = np.asarray(inputs["Wn"], dtype=np.float32)
    ln_scale = np.asarray(inputs["ln_scale"], dtype=np.float32)
    ln_bias = np.asarray(inputs["ln_bias"], dtype=np.float32)
    W1 = np.asarray(inputs["W1"], dtype=np.float32)
    b1 = np.asarray(inputs["b1"], dtype=np.float32)
    W2 = np.asarray(inputs["W2"], dtype=np.float32)
    b2 = np.asarray(inputs["b2"], dtype=np.float32)
    assert np.allclose(ln_bias, 0.0), "kernel assumes ln_bias == 0"

    C = (np.eye(HID) - np.ones((HID, HID)) / HID).astype(np.float32)
    Bstar = np.eye(HID, dtype=np.float32)
    for l in range(4):
        A = np.eye(HID, dtype=np.float32) + (Wn[l, 0] + Wn[l, 1]) * 0.5
        S = (
            np.diag(ln_scale[l - 1]).astype(np.float32)
            if l > 0 else np.eye(HID, dtype=np.float32)
        )
        Bstar = Bstar @ (S @ A @ C)
    Bstar = Bstar.astype(np.float32)
    W1p = (np.diag(ln_scale[3]).astype(np.float32) @ W1).astype(np.float32)

    BD = np.zeros((128, 128), np.float32)
    BD[0:64, 0:64] = Bstar
    BD[64:128, 64:128] = Bstar
    EO = np.zeros((128, 16), np.float32)
    EO[0:64, 0:8] = 1.0
    EO[64:128, 8:16] = 1.0
    cf = np.zeros((64, 35), np.float32)
    cf[:, 0:32] = W1p
    cf[0:32, 32] = b1
    cf[0:32, 33] = W2[:, 0]
    cf[0, 34] = b2[0]
    cf = np.ascontiguousarray(cf)

    bounds = np.searchsorted(batch, np.arange(0, 65, GPC))
    in_maps = []
    for c in range(NCORES):
        s, e = int(bounds[c]), int(bounds[c + 1])
        n = e - s
        assert n <= NPAD, f"core {c} shard {n} > NPAD {NPAD}"
        xp = np.zeros((NPAD, HID), np.float32)
        xp[:n] = x[s:e]
        xpr = xp.reshape(P, 2, HID)
        xT2 = np.concatenate([xpr[:, 0, :].T, xpr[:, 1, :].T], axis=0)
        xPW = (
            xpr.reshape(P, 128).reshape(PBLK, 128, 128)
            .transpose(1, 0, 2).reshape(128, PBLK * 128)
        )
        Mp = np.zeros((128, PBLK * 16), np.float32)
        i = np.arange(n)
        gb = (batch[s:e] - GPC * c).astype(np.int64)
        p = i // 2
        Mp[p % 128, (p // 128) * 16 + (i % 2) * 8 + gb] = 1.0
        cbm = np.concatenate([BD, Mp, EO], axis=1)
        cb8m = np.concatenate([EO, BD], axis=1)
        in_maps.append(
            dict(
                xT2=np.ascontiguousarray(xT2.astype(adt)),
                xPW=np.ascontiguousarray(xPW.astype(bf16)),
                cb=np.ascontiguousarray(cbm.astype(bf16)),
                cb8=np.ascontiguousarray(cb8m.astype(fp8)),
                cf=cf,
            )
        )
    return in_maps


def kernel(**inputs):
    global _prog
    from concourse import bass_utils

    in_maps = _prep_inputs(inputs)
    if _prog is None:
        _prog = _build_program()
    res = bass_utils.run_bass_kernel_spmd(
        _prog, in_maps, core_ids=list(range(NCORES))
    )
    outs = [np.asarray(res.results[c]["out"]).reshape(GPC) for c in range(NCORES)]
    return np.concatenate(outs).reshape(64, 1).astype(np.float32)


# revision 19
# speedup vs baseline: 1.9220x; 1.0907x over previous
"""Trainium2 Bass kernel for nn_MEGANCore (GATv2-style message-passing GNN).

Algebraic collapse (same as prior version): the reference's _gatv2 gathers
x_j = xp[col] and segment-sums x_j * alpha by col; softmax weights sum to 1
per segment, so aggregation == xp and the edges never matter.  With
ln_bias == 0 the 4-layer chain folds into one matrix B* (host-precomputed);
per-node LN scalars cancel except a final c4 = rsqrt(mean((x @ B*)^2)).
Since pooling is linear, g_b = (sum_n c4_n x_n) @ B*, so the device computes

    sumsq_n = ||x_n @ B*||^2        (A-phase + square + reduce)
    c4_n    = rsqrt(sumsq_n/64+eps)
    g0      = sum_n c4_n x_n        (pooling over raw x, per graph)
    out     = relu((g0@B*)@W1'+b1)@W2+b2

Device mapping (all x traffic bf16, ~0.85 MB per layout copy per core):
  A-phase : stationary block-diag [[B*,0],[0,B*]] (one FWL load), stream
            pair-major xT2[128, 3328] -> h~ for 2 nodes/cycle, PSUM [128,512]
  square  : PSUM->SBUF eviction split ACT/DVE, bf16 out
  reduce  : sq 128-col blocks as FWL weights x even/odd ones mask [128,16]
            -> per-pair sumsq lands node-major (transpose+reduce in one MM)
  pooling : xPW 128-col blocks as FWL weights x c4-weighted one-hot Q
            -> g0^T accumulated in PSUM [128,16] (even/odd feature halves)
  head    : two accumulating MMs on row-groups fold even/odd, then tiny MLP
  warmup  : dummy MMs + dummy activations during the DMA wait keep the PE
            HAM-warm (2.4 GHz) and hoist ACT table loads off the hot path
"""

import numpy as np

HID = 64
NCORES = 8
GPC = 8                  # graphs per core
NPAD = 6656              # padded nodes per core
P = NPAD // 2            # 3328 node-pairs
PBLK = P // 128          # 26 pair-blocks
CHUNK = 512              # pairs per A-phase matmul
NCH = (P + CHUNK - 1) // CHUNK   # 7 chunks (6x512 + 256)
EPS_SQ = 1e-9
FP8_A = True             # xT2 in fp8e4 (affects only the c4 stats path)
FP8_SQ = True            # squared activations in fp8e4 (halves reduce LDW)
NXDMA = 2                # xT2 arrives in this many column-chunks
# c4/pooling pieces (descending so the last piece's chain is short)
PIECES = [(0, 13), (13, 7), (20, 4), (24, 2)]   # (start block, nblocks)

_prog = None


def _build_program():
    import concourse.tile as tile
    from concourse import bacc, mybir
    from contextlib import ExitStack

    f32 = mybir.dt.float32
    f32r = mybir.dt.float32r
    bf16 = mybir.dt.bfloat16
    fp8 = mybir.dt.float8e4
    adt = fp8 if FP8_A else bf16

    nc = bacc.Bacc(
        "TRN2", target_bir_lowering=False, debug=False, num_devices=NCORES
    )
    xT2 = nc.dram_tensor("xT2", [128, P], adt, kind="ExternalInput").ap()
    xPW = nc.dram_tensor("xPW", [128, PBLK * 128], bf16, kind="ExternalInput").ap()
    cb = nc.dram_tensor("cb", [128, 560], bf16, kind="ExternalInput").ap()
    cb8 = nc.dram_tensor("cb8", [128, 144], fp8, kind="ExternalInput").ap()
    cf = nc.dram_tensor("cf", [65, 33], f32, kind="ExternalInput").ap()
    out = nc.dram_tensor("out", [1, GPC], f32, kind="ExternalOutput").ap()

    with tile.TileContext(nc) as tc:
        with ExitStack() as ctx:
            _body(ctx, tc, nc, mybir, xT2, xPW, cb, cb8, cf, out)
    nc.compile()
    return nc


def _body(ctx, tc, nc, mybir, xT2, xPW, cb, cb8, cf, out):
    f32 = mybir.dt.float32
    f32r = mybir.dt.float32r
    bf16 = mybir.dt.bfloat16
    fp8 = mybir.dt.float8e4
    adt = fp8 if FP8_A else bf16
    sdt = fp8 if FP8_SQ else bf16
    AF = mybir.ActivationFunctionType

    const = ctx.enter_context(tc.tile_pool(name="const", bufs=1))
    spool = ctx.enter_context(tc.tile_pool(name="scr", bufs=1))
    sqt = ctx.enter_context(tc.tile_pool(name="sqt", bufs=3))
    aps = ctx.enter_context(tc.tile_pool(name="aps", bufs=3, space="PSUM"))
    wps = ctx.enter_context(tc.tile_pool(name="wps", bufs=1, space="PSUM"))
    cpsp = ctx.enter_context(tc.tile_pool(name="cps", bufs=1, space="PSUM"))
    bpsp = ctx.enter_context(tc.tile_pool(name="bps", bufs=1, space="PSUM"))
    gps = ctx.enter_context(tc.tile_pool(name="gps", bufs=1, space="PSUM"))

    # ---- local scratch (no DMA deps) ----
    epsb = const.tile([128, 1], f32, tag="epsb")
    nc.vector.memset(epsb[:], EPS_SQ)
    scr = const.tile([128, 512], bf16, tag="scr")
    nc.vector.memset(scr[:], 0.0)
    dumm = const.tile([1, 4], f32, tag="dumm")
    nc.vector.memset(dumm[:], 1.0)

    # ---- input DMAs: sync-engine HWDGE executes FIFO, so issue in the
    # ---- order the data is needed: xT2 chunks, then xPW; consts on gpsimd
    xasb = spool.tile([128, P], adt, tag="xasb")
    xdw = P // NXDMA
    for d in range(NXDMA):
        nc.sync.dma_start(
            xasb[:, d * xdw:(d + 1) * xdw], xT2[:, d * xdw:(d + 1) * xdw]
        )
    xbsb = spool.tile([128, PBLK * 128], bf16, tag="xbsb")
    nc.sync.dma_start(xbsb[:], xPW)
    cbsb = const.tile([128, 560], bf16, tag="cbsb")
    nc.gpsimd.dma_start(cbsb[:], cb)
    cb8sb = const.tile([128, 144], fp8, tag="cb8sb")
    nc.gpsimd.dma_start(cb8sb[:], cb8)
    cfsb = const.tile([65, 33], f32, tag="cfsb")
    nc.gpsimd.dma_start(cfsb[:], cf)
    BD = cbsb[:, 0:128]
    Mp = cbsb[:, 128:544]
    EO = cb8sb[:, 0:16] if FP8_SQ else cbsb[:, 544:560]

    # ---- ACT table warm (hoists ACT_TABLE_LOAD off the critical path) ----
    dto = const.tile([1, 4], f32, tag="dto")
    nc.scalar.activation(dto[:, 0:1], dumm[:1, 0:1], AF.Square)
    nc.scalar.activation(dto[:, 1:2], dumm[:1, 1:2], AF.Sqrt, bias=epsb[0:1, :])
    nc.scalar.activation(dto[:, 2:3], dumm[:1, 2:3], AF.Relu, bias=epsb[0:1, :])

    # ---- PE HAM warmup: junk matmuls on zeroed scratch during DMA wait ----
    wp = wps.tile([128, 512], f32, tag="warm")
    for _ in range(4):
        nc.tensor.matmul(wp[:], scr[:, 0:128], scr[:], start=True, stop=True)

    sq = spool.tile([128, P], sdt, tag="sq")
    c4a = spool.tile([128, PBLK * 16], f32, tag="c4a")
    c4r = spool.tile([128, PBLK * 16], f32, tag="c4r")
    Q = spool.tile([128, PBLK * 16], bf16, tag="Q")
    cps = cpsp.tile([128, PBLK * 16], f32, tag="cps")
    bps = bpsp.tile([128, 16], f32, tag="bps")

    achunks = []
    for c in range(NCH):
        c0 = c * CHUNK
        w = min(CHUNK, P - c0)
        achunks.append((c0, w))

    def emit_A(c):
        c0, w = achunks[c]
        ps = aps.tile([128, 512], f32, tag="aps")
        nc.tensor.matmul(ps[:, 0:w], BD, xasb[:, c0:c0 + w], start=True, stop=True)
        return ps

    def emit_sq(c, ps):
        c0, w = achunks[c]
        aw = 3 * w // 4          # ACT squares 3/4 straight from PSUM
        dw = w - aw              # DVE: copy-out then square
        nc.scalar.activation(sq[:, c0:c0 + aw], ps[:, 0:aw], AF.Square)
        t = sqt.tile([128, 128], bf16, tag="sqt")
        nc.vector.tensor_copy(t[:, 0:dw], ps[:, aw:w])
        nc.vector.tensor_mul(sq[:, c0 + aw:c0 + w], t[:, 0:dw], t[:, 0:dw])

    def emit_reduce(b):
        nc.tensor.matmul(
            cps[:, b * 16:(b + 1) * 16],
            sq[:, b * 128:(b + 1) * 128], EO,
            start=True, stop=True,
        )

    def emit_c4(pi):
        b0, nb = PIECES[pi]
        lo, hi = b0 * 16, (b0 + nb) * 16
        nc.scalar.activation(
            c4a[:, lo:hi], cps[:, lo:hi], AF.Sqrt, bias=epsb[:], scale=1.0 / 64
        )
        nc.vector.reciprocal_approx_fast(c4r[:, lo:hi], c4a[:, lo:hi])
        nc.vector.tensor_mul(Q[:, lo:hi], Mp[:, lo:hi], c4r[:, lo:hi])

    def emit_B(pi):
        b0, nb = PIECES[pi]
        for b in range(b0, b0 + nb):
            nc.tensor.matmul(
                bps[:], xbsb[:, b * 128:(b + 1) * 128], Q[:, b * 16:(b + 1) * 16],
                start=(b == 0), stop=(b == PBLK - 1),
                skip_group_check=True,
            )

    # ---- software-pipelined emission ----
    # chunk c covers reduce blocks 4c..4c+3 (last chunk: 2 blocks)
    pend = {}
    pend[0] = emit_A(0)
    pend[1] = emit_A(1)
    emit_sq(0, pend.pop(0))
    for c in range(2, NCH):
        pend[c] = emit_A(c)
        emit_sq(c - 1, pend.pop(c - 1))
        for b in range(4 * (c - 2), 4 * (c - 1)):
            emit_reduce(b)
        if c == NCH - 1:            # blocks 0..15 emitted; piece 0 = 0..12
            emit_c4(0)
            emit_B(0)
    emit_sq(NCH - 1, pend.pop(NCH - 1))
    for b in range(4 * (NCH - 2), 4 * (NCH - 2) + 4):   # blocks 20..23
        emit_reduce(b)
    emit_c4(1)                                          # blocks 13..19
    emit_B(1)
    for b in range(24, PBLK):                           # blocks 24, 25
        emit_reduce(b)
    emit_c4(2)                                          # blocks 20..23
    emit_B(2)
    emit_c4(3)                                          # blocks 24, 25
    emit_B(3)

    # ---- evict pooled sums, then head: gT = B*^T (g0Te + g0To), MLP ----
    g0sb = spool.tile([128, 16], bf16, tag="g0sb")
    nc.vector.tensor_copy(g0sb[:], bps[:])
    gt = gps.tile([64, 8], f32, tag="gmlp")
    nc.tensor.matmul(gt[:], cbsb[0:64, 0:64], g0sb[0:64, 0:8],
                     start=True, stop=False)
    nc.tensor.matmul(gt[:], cbsb[64:128, 64:128], g0sb[64:128, 8:16],
                     start=False, stop=True)
    # gsb row 64 is a constant 1.0 so cf's b1 row rides the matmul
    gsb = spool.tile([65, 8], f32, tag="gsb")
    nc.vector.memset(gsb[64:65, :], 1.0)
    nc.vector.tensor_copy(gsb[0:64, :], gt[:])
    hid = gps.tile([32, 8], f32, tag="gmlp")
    nc.tensor.matmul(hid[:], cfsb[:, 0:32], gsb[:], start=True, stop=True)
    # hsb row 32 is a constant 1.0 so cf's b2 entry rides the matmul
    hsb = spool.tile([33, 8], f32, tag="hsb")
    nc.vector.memset(hsb[32:33, :], 1.0)
    nc.scalar.activation(hsb[0:32, :], hid[:], AF.Relu)
    o = gps.tile([1, 8], f32, tag="gmlp")
    nc.tensor.matmul(o[:], cfsb[0:33, 32:33], hsb[:], start=True, stop=True)
    osb = spool.tile([1, 8], f32, tag="osb")
    nc.vector.tensor_copy(osb[:], o[:])
    nc.sync.dma_start(out, osb[:])


def _prep_inputs(inputs):
    import ml_dtypes

    bf16 = ml_dtypes.bfloat16
    fp8 = ml_dtypes.float8_e4m3fn
    adt = fp8 if FP8_A else bf16
    x = np.ascontiguousarray(np.asarray(inputs["x"], dtype=np.float32))
    batch = np.asarray(inputs["batch"]).astype(np.int64)
    Wn = np.asarray(inputs["Wn"], dtype=np.float32)
    ln_scale = np.asarray(inputs["ln_scale"], dtype=np.float32)
    ln_bias = np.asarray(inputs["ln_bias"], dtype=np.float32)
    W1 = np.asarray(inputs["W1"], dtype=np.float32)
    b1 = np.asarray(inputs["b1"], dtype=np.float32)
    W2 = np.asarray(inputs["W2"], dtype=np.float32)
    b2 = np.asarray(inputs["b2"], dtype=np.float32)
    assert np.allclose(ln_bias, 0.0), "kernel assumes ln_bias == 0"

    C = (np.eye(HID) - np.ones((HID, HID)) / HID).astype(np.float32)
    Bstar = np.eye(HID, dtype=np.float32)
    for l in range(4):
        A = np.eye(HID, dtype=np.float32) + (Wn[l, 0] + Wn[l, 1]) * 0.5
        S = (
            np.diag(ln_scale[l - 1]).astype(np.float32)
            if l > 0 else np.eye(HID, dtype=np.float32)
        )
        Bstar = Bstar @ (S @ A @ C)
    Bstar = Bstar.astype(np.float32)
    W1p = (np.diag(ln_scale[3]).astype(np.float32) @ W1).astype(np.float32)

    BD = np.zeros((128, 128), np.float32)
    BD[0:64, 0:64] = Bstar
    BD[64:128, 64:128] = Bstar
    EO = np.zeros((128, 16), np.float32)
    EO[0:64, 0:8] = 1.0
    EO[64:128, 8:16] = 1.0
    cf = np.zeros((65, 33), np.float32)
    cf[0:64, 0:32] = W1p
    cf[64, 0:32] = b1           # rides on gsb's constant-1 row
    cf[0:32, 32] = W2[:, 0]
    cf[32, 32] = b2[0]          # rides on hsb's constant-1 row
    cf = np.ascontiguousarray(cf)

    bounds = np.searchsorted(batch, np.arange(0, 65, GPC))
    in_maps = []
    for c in range(NCORES):
        s, e = int(bounds[c]), int(bounds[c + 1])
        n = e - s
        assert n <= NPAD, f"core {c} shard {n} > NPAD {NPAD}"
        xp = np.zeros((NPAD, HID), np.float32)
        xp[:n] = x[s:e]
        xpr = xp.reshape(P, 2, HID)
        xT2 = np.concatenate([xpr[:, 0, :].T, xpr[:, 1, :].T], axis=0)
        xPW = (
            xpr.reshape(P, 128).reshape(PBLK, 128, 128)
            .transpose(1, 0, 2).reshape(128, PBLK * 128)
        )
        Mp = np.zeros((128, PBLK * 16), np.float32)
        i = np.arange(n)
        gb = (batch[s:e] - GPC * c).astype(np.int64)
        p = i // 2
        Mp[p % 128, (p // 128) * 16 + (i % 2) * 8 + gb] = 1.0
        cbm = np.concatenate([BD, Mp, EO], axis=1)
        cb8m = np.concatenate([EO, BD], axis=1)
        in_maps.append(
            dict(
                xT2=np.ascontiguousarray(xT2.astype(adt)),
                xPW=np.ascontiguousarray(xPW.astype(bf16)),
                cb=np.ascontiguousarray(cbm.astype(bf16)),
                cb8=np.ascontiguousarray(cb8m.astype(fp8)),
                cf=cf,
            )
        )
    return in_maps


def kernel(**inputs):
    global _prog
    from concourse import bass_utils

    in_maps = _prep_inputs(inputs)
    if _prog is None:
        _prog = _build_program()
    res = bass_utils.run_bass_kernel_spmd(
        _prog, in_maps, core_ids=list(range(NCORES))
    )
    outs = [np.asarray(res.results[c]["out"]).reshape(GPC) for c in range(NCORES)]
    return np.concatenate(outs).reshape(64, 1).astype(np.float32)
